# revision 1
# baseline (speedup 1.0000x reference)
"""Trainium2 Bass kernel for nn_Attention1 (dense transformer attention with
query-summed output).

Reference computation (per batch b):
    query  = x * drop_mask                       [S, D]
    scores = query @ x.T / sqrt(D)               [S, S]
    att    = softmax(scores, axis=-1)
    out[b] = (att @ x).sum(axis=queries)         [D]

Key identity: out[b] = w @ x where w[k] = sum_q att[q, k] (attention column
sums), so the full PV matmul is never needed — only the column sums of the
softmax matrix.

Sharding: pure data parallel, batch B=8 across the 8 NeuronCores (one batch
element per core).

Per-core algorithm (S=4096, D=256):
  1. Load x, mask (f32); build q16 = fp16(x * mask / 16), x16 = fp16(x);
     per-row bias c_q = -(diag score) + 8*ln2 (keeps exp() outputs centered
     in fp16 range: diagonal -> 256, no overflow/underflow of meaningful mass).
  2. Transpose q16/x16 to [D, S] layout: bounce through DRAM, then XBAR
     DMA-transpose loads ([512,128] -> [128,512]).
  3. For each 128-query stripe: scores tile = qT.T @ xT on PE (fp16 in, f32
     PSUM), then ScalarE computes e = exp(scores + c_q) -> fp16 SBUF with
     fused per-row accumulation (row sums Z).
  4. r_q = fp16(1/Z_q) (VectorE). Per 512-query block, weighted column sums
     w[k] += sum_q r_q * e[q, k] via M=1 matmuls with r as the stationary.
  5. w (a [1, 4096] row on one partition) is transposed into partitions with
     K=1 matmuls against a ones-scalar, then out = w @ x with 32 accumulating
     matmuls over 128-row chunks of x. DMA out.
"""

import os
import sys

import numpy as np

_TRN_REPO = "/opt/trn_rl_repo"
if os.path.isdir(_TRN_REPO) and _TRN_REPO not in sys.path:
    sys.path.insert(0, _TRN_REPO)

import concourse.bass as bass
import concourse.mybir as mybir
import concourse.tile as tile
from concourse import bacc
from concourse.bass_utils import run_bass_kernel_spmd

F32 = mybir.dt.float32
F16 = mybir.dt.float16

B = 8
S = 4096
D = 256
P = 128

NST = S // P          # 32 query stripes of 128 rows
NSB = S // 512        # 8 blocks of 512 rows (4 stripes)
E_SHIFT = float(8 * np.log(2.0))  # exp() output centering: diagonal -> 2^8
INV_SQRT_D = 1.0 / 16.0

# k-slices for the scores PSUM tiles: 1536-wide tiles (3 banks each, 2 bufs)
K_SLICES = [(0, 1536), (1536, 1536), (3072, 1024)]


def build_kernel(finalize: bool = True) -> bass.Bass:
    # Bacc (not plain Bass): its compile pipeline splits multi-sem waits into
    # event semaphores, which the TRN2 DMA pseudo-instructions require.
    nc = bacc.Bacc(None)

    x_in = nc.declare_dram_parameter("x", [S, D], F32, isOutput=False)
    m_in = nc.declare_dram_parameter("mask", [S, D], F32, isOutput=False)
    out_ext = nc.declare_dram_parameter("out", [1, D], F32, isOutput=True)

    # DRAM views grouping rows into [stripe, partition, d] for batched loads
    x_in_t = x_in.rearrange("(a p) d -> p a d", p=P)      # [128, 32, 256]
    m_in_t = m_in.rearrange("(a p) d -> p a d", p=P)

    with tile.TileContext(nc) as tc:
        with (
            tc.tile_pool(name="dram", bufs=1, space="DRAM") as dramp,
            tc.tile_pool(name="resident", bufs=1) as res,
            tc.tile_pool(name="stage", bufs=3) as stage,
            tc.tile_pool(name="etile", bufs=8) as ep,
            tc.tile_pool(name="zp", bufs=4) as zpp,
            tc.tile_pool(name="small", bufs=8) as smallp,
            tc.tile_pool(name="r16", bufs=8) as rp,
            tc.tile_pool(name="ps_scores", bufs=2, space="PSUM") as pss,
            tc.tile_pool(name="ps_misc", bufs=2, space="PSUM") as psm,
        ):
            # DRAM bounce buffers for the XBAR transpose
            q16d = dramp.tile([S, D], F16)
            x16d = dramp.tile([S, D], F16)

            # SBUF residents
            xf = res.tile([P, NST, D], F32)        # x, f32, [partition, stripe, d]
            qT = res.tile([P, 2, S], F16)          # query^T/16, [d%128, d//128, s]
            xT = res.tile([P, 2, S], F16)          # x^T, same layout
            bias_all = res.tile([P, NST], F32)     # -diag/16 + E_SHIFT per row
            wsum = res.tile([1, S], F32)           # accumulated column sums
            ones1 = res.tile([1, 1], F32)
            wtot_sb = res.tile([P, NST], F32)      # w reshaped [k%128, k//128]
            out_sb = res.tile([1, D], F32)

            nc.vector.memset(ones1[:], 1.0)

            # ---- Phase A: load, cast, per-row bias, bounce, transpose ----
            for sb in range(NSB):
                a0 = sb * 4  # first stripe of this 512-row block
                rows = slice(sb * 512, (sb + 1) * 512)

                # batched loads: 512 rows at once
                nc.scalar.dma_start(xf[:, a0 : a0 + 4, :], x_in_t[:, a0 : a0 + 4, :])
                mk = stage.tile([P, 4, D], F32, tag="mask")
                nc.scalar.dma_start(mk[:], m_in_t[:, a0 : a0 + 4, :])

                # q16 = (x/16)*mask ; x16 = fp16(x)
                q16 = stage.tile([P, 4, D], F16, tag="q16")
                nc.vector.scalar_tensor_tensor(
                    out=q16[:],
                    in0=xf[:, a0 : a0 + 4, :],
                    scalar=INV_SQRT_D,
                    in1=mk[:],
                    op0=mybir.AluOpType.mult,
                    op1=mybir.AluOpType.mult,
                )
                x16 = stage.tile([P, 4, D], F16, tag="x16")
                nc.vector.tensor_copy(x16[:], xf[:, a0 : a0 + 4, :])

                # diag(scores)/16 per row: t = q16*x, reduce innermost
                t_blk = stage.tile([P, 4, D], F32, tag="prod")
                nc.vector.tensor_tensor(
                    t_blk[:], q16[:], xf[:, a0 : a0 + 4, :], mybir.AluOpType.mult
                )
                nc.vector.tensor_reduce(
                    bias_all[:, a0 : a0 + 4],
                    t_blk[:],
                    mybir.AxisListType.X,
                    mybir.AluOpType.add,
                )
                # bias = E_SHIFT - diag/16
                nc.vector.tensor_scalar(
                    bias_all[:, a0 : a0 + 4],
                    bias_all[:, a0 : a0 + 4],
                    -1.0,
                    E_SHIFT,
                    mybir.AluOpType.mult,
                    mybir.AluOpType.add,
                )

                # bounce to DRAM for the XBAR transpose
                nc.scalar.dma_start(
                    q16d.rearrange("(a p) d -> p a d", p=P)[:, a0 : a0 + 4, :], q16[:]
                )
                nc.scalar.dma_start(
                    x16d.rearrange("(a p) d -> p a d", p=P)[:, a0 : a0 + 4, :], x16[:]
                )

                # XBAR transpose loads: [512, 128] -> [128, 512]
                for d in range(2):
                    nc.sync.dma_start(
                        qT[:, d, rows],
                        q16d[rows, d * P : (d + 1) * P],
                        transpose=True,
                    )
                    nc.sync.dma_start(
                        xT[:, d, rows],
                        x16d[rows, d * P : (d + 1) * P],
                        transpose=True,
                    )

            # ---- Phase B: scores -> exp -> row sums -> column sums ----
            def emit_colsum(blk, e_tiles, r_tiles):
                # weighted column sums for a block: w[k] += sum_q r_q e[q,k].
                # M=1 matvecs waste the PE array; run 4 concurrently on
                # disjoint 32-wide column strips (tile_position col groups),
                # outputs at partitions 0/32/64/96 of one PSUM bank.
                for g in range(2):
                    pw = psm.tile([P, 512], F32, tag="w")
                    # j outer / strip inner: consecutive matmuls target
                    # different column strips, so each strip's LDWEIGHTS can
                    # overlap the previous strip's matmul (per-subarray
                    # concurrency) instead of serializing on one strip.
                    for j in range(4):
                        for c in range(4):
                            ks = g * 4 + c
                            nc.tensor.matmul(
                                pw[32 * c : 32 * c + 1, :],
                                lhsT=r_tiles[j][:],
                                rhs=e_tiles[j][:, ks * 512 : (ks + 1) * 512],
                                start=(j == 0),
                                stop=(j == 3),
                                tile_position=(0, 32 * c),
                            )
                    for c in range(4):
                        ks = g * 4 + c
                        sl = slice(ks * 512, (ks + 1) * 512)
                        if blk == 0:
                            nc.vector.tensor_copy(
                                wsum[:, sl], pw[32 * c : 32 * c + 1, :]
                            )
                        else:
                            nc.vector.tensor_tensor(
                                wsum[:, sl],
                                wsum[:, sl],
                                pw[32 * c : 32 * c + 1, :],
                                mybir.AluOpType.add,
                            )

            prev = None
            for blk in range(NSB):
                e_tiles = []
                r_tiles = []
                for j in range(4):
                    qs = blk * 4 + j
                    et = ep.tile([P, S], F16, tag="e")
                    zp = zpp.tile([P, 4], F32, tag="z")
                    for ksl, (k0, kn) in enumerate(K_SLICES):
                        ps = pss.tile([P, 1536], F32, tag="s")
                        for d in range(2):
                            for n in range(kn // 512):
                                nc.tensor.matmul(
                                    ps[:, n * 512 : (n + 1) * 512],
                                    lhsT=qT[:, d, qs * P : (qs + 1) * P],
                                    rhs=xT[:, d, k0 + n * 512 : k0 + (n + 1) * 512],
                                    start=(d == 0),
                                    stop=(d == 1),
                                )
                        nc.scalar.activation(
                            out=et[:, k0 : k0 + kn],
                            in_=ps[:, :kn],
                            func=mybir.ActivationFunctionType.Exp,
                            bias=bias_all[:, qs : qs + 1],
                            scale=1.0,
                            accum_out=zp[:, ksl : ksl + 1],
                        )
                    # Z = sum of the 3 partial row sums; r = fp16(1/Z)
                    zs = smallp.tile([P, 1], F32, tag="zs")
                    nc.vector.tensor_tensor(
                        zs[:], zp[:, 0:1], zp[:, 1:2], mybir.AluOpType.add
                    )
                    nc.vector.tensor_tensor(
                        zs[:], zs[:], zp[:, 2:3], mybir.AluOpType.add
                    )
                    nc.vector.reciprocal(zs[:], zs[:])
                    rt = rp.tile([P, 1], F16, tag="r")
                    nc.vector.tensor_copy(rt[:], zs[:])
                    e_tiles.append(et)
                    r_tiles.append(rt)
                    # defer the previous block's column sums until the current
                    # block's first stripe is emitted, so ScalarE always has a
                    # scores tile queued while PE runs the column-sum matvecs
                    if j == 0 and prev is not None:
                        emit_colsum(blk - 1, *prev)
                        prev = None
                prev = (e_tiles, r_tiles)
            emit_colsum(NSB - 1, *prev)

            # ---- Phase C: transpose w into partitions, final matvec ----
            # wtotP[p, c] = w[c*128 + p], built with K=1 matmuls (row->column)
            wtotP = psm.tile([P, NST], F32, tag="w")
            for c in range(NST):
                nc.tensor.matmul(
                    wtotP[:, c : c + 1],
                    lhsT=wsum[:, c * P : (c + 1) * P],
                    rhs=ones1[:],
                    start=True,
                    stop=True,
                )
            nc.vector.tensor_copy(wtot_sb[:], wtotP[:])

            # out[1, D] = sum_c wtot[:, c]^T @ x[c*128:(c+1)*128, :]
            po = psm.tile([1, D], F32, tag="w")
            for c in range(NST):
                nc.tensor.matmul(
                    po[:],
                    lhsT=wtot_sb[:, c : c + 1],
                    rhs=xf[:, c, :],
                    start=(c == 0),
                    stop=(c == NST - 1),
                )
            nc.scalar.copy(out_sb[:], po[:])
            nc.sync.dma_start(out_ext[:, :], out_sb[:])

    if finalize:
        nc.finalize()
    return nc


def _run(x: np.ndarray, drop_mask: np.ndarray, trace: bool = False, nc=None):
    if nc is None:
        nc = build_kernel()
    in_maps = [{"x": x[b], "mask": drop_mask[b]} for b in range(B)]
    res = run_bass_kernel_spmd(nc, in_maps, list(range(B)), trace=trace)
    out = np.stack([res.results[b]["out"].reshape(D) for b in range(B)])
    return out.astype(np.float32), res


def kernel(**inputs: np.ndarray) -> np.ndarray:
    x = np.ascontiguousarray(inputs["x"], dtype=np.float32)
    drop_mask = np.ascontiguousarray(inputs["drop_mask"], dtype=np.float32)
    assert x.shape == (B, S, D) and drop_mask.shape == (B, S, D)
    out, _ = _run(x, drop_mask)
    return out


def profile(**inputs: np.ndarray):
    x = np.ascontiguousarray(inputs["x"], dtype=np.float32)
    drop_mask = np.ascontiguousarray(inputs["drop_mask"], dtype=np.float32)
    out, res = _run(x, drop_mask, trace=True)
    return res.exec_time_ns


if __name__ == "__main__":
    rng = np.random.default_rng(0)
    x = rng.standard_normal((B, S, D)).astype(np.float32)
    m = (rng.random((B, S, D)) < 0.5).astype(np.float32) * 2.0
    out = kernel(x=x, drop_mask=m)
    print(out.shape, out.dtype)

